# revision 1
# baseline (speedup 1.0000x reference)
"""LLaMA-style MLP (gate/up/silu/down) on 8 Trainium2 NeuronCores.

Strategy: data-parallel over tokens (8192 tokens -> 1024/core), fp8
matmuls in DoubleRow perf mode (2 fp8 contraction elements per PE pass,
0.5 cycles/row -> 4x bf16 matmul throughput) with fp32 PSUM
accumulation and no collectives.

Accuracy is recovered with an error-compensated hi/lo split: every
operand V is represented as V ~= V_hi + V_lo with both parts in
float8_e4m3 (V_hi = fp8(V), V_lo = fp8(V - V_hi)).  Each logical
matmul W@X becomes three fp8 DoubleRow matmuls accumulated in PSUM:

    W@X ~= W_hi@X_hi + W_hi@X_lo + W_lo@X_hi     (lo*lo term dropped)

at 3 * 0.25 = 0.75x the cycle cost of one bf16 matmul.  Measured
end-to-end relative error of this scheme is ~3e-3 (vs 2e-2 budget).

Weights are pre-scaled by 64 on the host so their magnitudes sit in
e4m3's normal range (0.02*randn would otherwise quantize terribly);
the scale is divided back out on-device (SiLU input scale, down-proj
output scale).

Layouts (host pre-permutes; partition dim first, contraction subtiles
paired for DoubleRow's [p, 2, free] operand shape):

  x   -> xh/xl [n_tn, 128, D/128, TB]    x*[tn,p,k,t] = fp8 split of x[tok, k*128+p]
  Wg  -> wgh/wgl [F/128, 128, D/128, 128]  = fp8 split of 64*Wg[fm*128+m, k*128+p]
  Wu  -> wuh/wul (same layout)
  Wd  -> wdh/wdl [D/128, 128, F/128, 128]  = fp8 split of 64*Wd[dm*128+m, k*128+p]
  out <- y [D/128, 128, T] f32           y[dm,p,t] = out[tok, dm*128+p]

Per 512-token block: gate/up PSUM chains contract D in 16 pairs x 3
terms, SiLU(psg/64) on the scalar engine, h = (psu/64)*sg fused on the
vector engine (fp32), then h is split to fp8 hi/lo on-device and the
down projection contracts F in 43 pairs x 3 terms.

Overlap details that buy the last ~7us:
 - The first slab's chains are emitted term-major (all (Wh,xh) chunks,
   then (Wl,xh), then (Wh,xl)) with the warmup DMAs interleaved in the
   same order, so the PE starts as soon as Wh+xh land instead of
   waiting for all four operand tiles.
 - Dummy DoubleRow matmuls on memset scratch keep the PE continuously
   busy across the initial DMA window and the in-flight-x_lo gap
   (N_WARM0/N_WARM1): the tensor engine's p-state ramp (half speed for
   3us after any idle) is spent on free work, never on real chains.
 - The next token block's x tiles are prefetched behind the first wd
   pair of the previous down phase.
 - The kernel's final down chain is split into four quarter-token
   chains so y copies/stores overlap the remaining matmuls instead of
   trailing the kernel.
"""

import os
import sys

sys.path.insert(0, "/opt/trn_rl_repo")

from contextlib import ExitStack

import numpy as np
import ml_dtypes

import concourse.bass as bass  # noqa: F401
import concourse.tile as tile
import concourse.mybir as mybir
from concourse import bacc
from concourse.bass_utils import run_bass_kernel_spmd

BF16 = mybir.dt.bfloat16
F32 = mybir.dt.float32
FP8 = mybir.dt.float8e4
NP_FP8 = ml_dtypes.float8_e4m3

# Problem shape (hardcoded per the task contract).
B, S, D, F = 4, 2048, 4096, 11008
N_CORES = 8
T_CORE = (B * S) // N_CORES  # tokens per core
TB = 512                     # token block (one PSUM bank of fp32)
W_SCALE = 64.0               # host-side weight scale (power of 2)

# Warmup PE-priming: dummy DoubleRow matmuls on memset scratch keep the
# tensor engine continuously busy while the first DMAs land, so the p-state
# ramp (2x-slow first 3us after any idle) is spent on free work, not real
# chains.  Counts tuned against the timeline sim.
N_WARM0 = 280  # kernel start -> first real matmul
N_WARM1 = 48   # last-operand gap inside the first term-major chain

DR = mybir.MatmulPerfMode.DoubleRow

LAST_RUN = {}


def build_module(T=T_CORE, tb=TB, d=D, f=F):
    """Build the single-core Bass module (same program on all 8 cores)."""
    n_tn = T // tb          # 2 token blocks
    n_dk = d // 128         # 32 contraction subtiles for gate/up
    n_fm = f // 128         # 86 F slabs
    n_fk = f // 128         # 86 contraction subtiles for down
    n_dm = d // 128         # 32 D slabs

    nc = bacc.Bacc("TRN2", target_bir_lowering=False, debug=False)
    xh = nc.dram_tensor("xh", [n_tn, 128, n_dk, tb], FP8, kind="ExternalInput").ap()
    xl = nc.dram_tensor("xl", [n_tn, 128, n_dk, tb], FP8, kind="ExternalInput").ap()
    wgh = nc.dram_tensor("wgh", [n_fm, 128, n_dk, 128], FP8, kind="ExternalInput").ap()
    wgl = nc.dram_tensor("wgl", [n_fm, 128, n_dk, 128], FP8, kind="ExternalInput").ap()
    wuh = nc.dram_tensor("wuh", [n_fm, 128, n_dk, 128], FP8, kind="ExternalInput").ap()
    wul = nc.dram_tensor("wul", [n_fm, 128, n_dk, 128], FP8, kind="ExternalInput").ap()
    wdh = nc.dram_tensor("wdh", [n_dm, 128, n_fk, 128], FP8, kind="ExternalInput").ap()
    wdl = nc.dram_tensor("wdl", [n_dm, 128, n_fk, 128], FP8, kind="ExternalInput").ap()
    y = nc.dram_tensor("y", [n_dm, 128, T], F32, kind="ExternalOutput").ap()

    inv = 1.0 / W_SCALE

    with tile.TileContext(nc) as tc, ExitStack() as ctx:
        xpool = ctx.enter_context(tc.tile_pool(name="x", bufs=1))
        wpool = ctx.enter_context(tc.tile_pool(name="w", bufs=2))
        wdpool = ctx.enter_context(tc.tile_pool(name="wdp", bufs=2))
        hpool = ctx.enter_context(tc.tile_pool(name="h", bufs=1))
        spool = ctx.enter_context(tc.tile_pool(name="s", bufs=2))
        fpool = ctx.enter_context(tc.tile_pool(name="hf", bufs=2))
        ypool = ctx.enter_context(tc.tile_pool(name="y", bufs=2))
        psum = ctx.enter_context(tc.tile_pool(name="psum", bufs=4, space="PSUM"))
        psumy = ctx.enter_context(tc.tile_pool(name="psumy", bufs=2, space="PSUM"))
        zpool = ctx.enter_context(tc.tile_pool(name="z", bufs=1))

        n_pairs = n_dk // 2

        zw_sb = zpool.tile([128, 2, 128], FP8, tag="zw")
        nc.vector.memset(zw_sb[:], 0)
        zx_sb = zpool.tile([128, 2, 128], FP8, tag="zx")
        nc.vector.memset(zx_sb[:], 0)

        def emit_warm(n):
            """n small dummy DoubleRow matmuls into scratch PSUM (borrows a
            psy-tagged bank, which is otherwise idle during warmup)."""
            if n <= 0:
                return
            psz = psumy.tile([128, tb], F32, tag="psy")
            for i in range(n):
                nc.tensor.matmul(
                    psz[:, 0:128], zw_sb[:], zx_sb[:],
                    start=(i == 0), stop=(i == n - 1), perf_mode=DR,
                )

        def emit_proj(ps, wh_sb, wl_sb, xh_sb, xl_sb, term_major, warm_fill=0):
            """One 3-term DoubleRow accumulation chain into `ps`.

            chunk-major needs all 4 operand tiles before the first matmul;
            term-major orders the groups (Wh,xh), (Wl,xh), (Wh,xl) so the
            chain can start as soon as Wh and xh have landed — used for the
            first slab, where the chain start gates on the initial DMAs.
            """
            if term_major == "wl_last":
                # W_lo is the last operand to land for this chain
                groups = [(wh_sb, xh_sb), (wh_sb, xl_sb), (wl_sb, xh_sb)]
            else:
                # x_lo is the last operand to land
                groups = [(wh_sb, xh_sb), (wl_sb, xh_sb), (wh_sb, xl_sb)]
            if term_major:
                for gi, (w_sb, x_sb) in enumerate(groups):
                    if gi == 2:
                        # the last operand's transfer may still be in flight;
                        # busy-wait on scratch so the p-state ramp isn't reset
                        emit_warm(warm_fill)
                    for c in range(n_pairs):
                        sl = slice(2 * c, 2 * c + 2)
                        nc.tensor.matmul(
                            ps[:], w_sb[:, sl], x_sb[:, sl],
                            start=(gi == 0 and c == 0),
                            stop=(gi == 2 and c == n_pairs - 1),
                            perf_mode=DR,
                        )
            else:
                for c in range(n_pairs):
                    sl = slice(2 * c, 2 * c + 2)
                    nc.tensor.matmul(
                        ps[:], wh_sb[:, sl], xh_sb[:, sl],
                        start=(c == 0), stop=False, perf_mode=DR,
                    )
                    nc.tensor.matmul(
                        ps[:], wh_sb[:, sl], xl_sb[:, sl],
                        start=False, stop=False, perf_mode=DR,
                    )
                    nc.tensor.matmul(
                        ps[:], wl_sb[:, sl], xh_sb[:, sl],
                        start=False, stop=(c == n_pairs - 1), perf_mode=DR,
                    )

        next_x = None
        for tn in range(n_tn):
            first = tn == 0
            if first:
                # Interleave the first slab's weight DMAs with the x DMAs in
                # the order the term-major chain consumes them.
                wgh_sb = wpool.tile([128, n_dk, 128], FP8, tag="wgh")
                nc.sync.dma_start(wgh_sb[:], wgh[0])
                xh_sb = xpool.tile([128, n_dk, tb], FP8, tag="xh")
                hk = n_dk // 2
                nc.sync.dma_start(xh_sb[:, :hk], xh[tn, :, :hk])
                nc.sync.dma_start(xh_sb[:, hk:], xh[tn, :, hk:])
                wgl_sb = wpool.tile([128, n_dk, 128], FP8, tag="wgl")
                nc.sync.dma_start(wgl_sb[:], wgl[0])
                xl_sb = xpool.tile([128, n_dk, tb], FP8, tag="xl")
                nc.sync.dma_start(xl_sb[:, :hk], xl[tn, :, :hk])
                nc.sync.dma_start(xl_sb[:, hk:], xl[tn, :, hk:])
                wuh_sb = wpool.tile([128, n_dk, 128], FP8, tag="wuh")
                nc.sync.dma_start(wuh_sb[:], wuh[0])
                wul_sb = wpool.tile([128, n_dk, 128], FP8, tag="wul")
                nc.sync.dma_start(wul_sb[:], wul[0])
                w0 = (wgh_sb, wgl_sb, wuh_sb, wul_sb)
                emit_warm(N_WARM0)
            else:
                # Issued during the previous block's down projection so the
                # transfers overlap stage B instead of stalling this block.
                xh_sb, xl_sb = next_x

            hh_sb = hpool.tile([128, n_fm, tb], FP8, tag="hh")
            hl_sb = hpool.tile([128, n_fm, tb], FP8, tag="hl")

            # Stage A: gate/up projection + silu + mul + fp8 split, one
            # 128-row slab of F at a time.
            for fm in range(n_fm):
                if first and fm == 0:
                    wgh_sb, wgl_sb, wuh_sb, wul_sb = w0
                else:
                    wgh_sb = wpool.tile([128, n_dk, 128], FP8, tag="wgh")
                    nc.sync.dma_start(wgh_sb[:], wgh[fm])
                    wgl_sb = wpool.tile([128, n_dk, 128], FP8, tag="wgl")
                    nc.sync.dma_start(wgl_sb[:], wgl[fm])
                    wuh_sb = wpool.tile([128, n_dk, 128], FP8, tag="wuh")
                    nc.sync.dma_start(wuh_sb[:], wuh[fm])
                    wul_sb = wpool.tile([128, n_dk, 128], FP8, tag="wul")
                    nc.sync.dma_start(wul_sb[:], wul[fm])

                tm = first and fm == 0
                psg = psum.tile([128, tb], F32, tag="ps")
                emit_proj(psg, wgh_sb, wgl_sb, xh_sb, xl_sb, tm,
                          warm_fill=N_WARM1 if tm else 0)
                psu = psum.tile([128, tb], F32, tag="ps")
                emit_proj(psu, wuh_sb, wul_sb, xh_sb, xl_sb,
                          "wl_last" if tm else False)

                # sg = silu(psg/64)  [bf16]
                sg = spool.tile([128, tb], BF16, tag="sg")
                nc.scalar.activation(
                    sg[:], psg[:], mybir.ActivationFunctionType.Silu, scale=inv
                )
                # h = (psu/64) * sg  [f32], then split to fp8 hi/lo
                hf = fpool.tile([128, tb], F32, tag="hf")
                nc.vector.scalar_tensor_tensor(
                    hf[:], psu[:], inv, sg[:],
                    mybir.AluOpType.mult, mybir.AluOpType.mult,
                )
                nc.vector.tensor_copy(hh_sb[:, fm], hf[:])
                nc.vector.tensor_sub(hl_sb[:, fm], hf[:], hh_sb[:, fm])

            # Stage B: down projection, contracting over all of F.
            for dm in range(n_dm):
                wdh_sb = wdpool.tile([128, n_fk, 128], FP8, tag="wdh")
                nc.sync.dma_start(wdh_sb[:], wdh[dm])
                wdl_sb = wdpool.tile([128, n_fk, 128], FP8, tag="wdl")
                nc.sync.dma_start(wdl_sb[:], wdl[dm])
                if dm == 1 and tn + 1 < n_tn:
                    # Prefetch the next token block's x behind the first wd
                    # pair (not ahead of it: that would head-of-line-block
                    # this stage's own start).
                    nxh = xpool.tile([128, n_dk, tb], FP8, tag="xh")
                    nc.sync.dma_start(nxh[:], xh[tn + 1])
                    nxl = xpool.tile([128, n_dk, tb], FP8, tag="xl")
                    nc.sync.dma_start(nxl[:], xl[tn + 1])
                    next_x = (nxh, nxl)
                n_fpairs = n_fk // 2
                if tn == n_tn - 1 and dm == n_dm - 1:
                    # Last chain of the kernel: split into four quarter-token
                    # chains so earlier quarters' y copy + store overlap the
                    # later quarters' matmuls instead of trailing the kernel.
                    for half in range(4):
                        ts = slice(half * (tb // 4), (half + 1) * (tb // 4))
                        psyh = psumy.tile([128, tb // 4], F32, tag="psyh")
                        for c in range(n_fpairs):
                            sl = slice(2 * c, 2 * c + 2)
                            nc.tensor.matmul(
                                psyh[:], wdh_sb[:, sl], hh_sb[:, sl, ts],
                                start=(c == 0), stop=False, perf_mode=DR,
                            )
                            nc.tensor.matmul(
                                psyh[:], wdh_sb[:, sl], hl_sb[:, sl, ts],
                                start=False, stop=False, perf_mode=DR,
                            )
                            nc.tensor.matmul(
                                psyh[:], wdl_sb[:, sl], hh_sb[:, sl, ts],
                                start=False, stop=(c == n_fpairs - 1),
                                perf_mode=DR,
                            )
                        y_sb = ypool.tile([128, tb // 4], F32, tag="yh")
                        nc.vector.tensor_scalar_mul(y_sb[:], psyh[:], inv)
                        nc.sync.dma_start(
                            y[dm, :, tn * tb + half * (tb // 4):
                                     tn * tb + (half + 1) * (tb // 4)],
                            y_sb[:])
                    continue
                psy = psumy.tile([128, tb], F32, tag="psy")
                for c in range(n_fpairs):
                    sl = slice(2 * c, 2 * c + 2)
                    nc.tensor.matmul(
                        psy[:], wdh_sb[:, sl], hh_sb[:, sl],
                        start=(c == 0), stop=False, perf_mode=DR,
                    )
                    nc.tensor.matmul(
                        psy[:], wdh_sb[:, sl], hl_sb[:, sl],
                        start=False, stop=False, perf_mode=DR,
                    )
                    nc.tensor.matmul(
                        psy[:], wdl_sb[:, sl], hh_sb[:, sl],
                        start=False, stop=(c == n_fpairs - 1), perf_mode=DR,
                    )
                y_sb = ypool.tile([128, tb], F32, tag="y")
                nc.vector.tensor_scalar_mul(y_sb[:], psy[:], inv)
                nc.sync.dma_start(y[dm, :, tn * tb:(tn + 1) * tb], y_sb[:])

    nc.compile()
    return nc


def _fp8_split(a):
    """Split float32 array into (hi, lo) float8_e4m3 parts."""
    hi = a.astype(NP_FP8)
    lo = (a - hi.astype(np.float32)).astype(NP_FP8)
    return hi, lo


def _prep_inputs(x, W_gate, W_up, W_down, T=T_CORE, tb=TB, d=D, f=F,
                 n_cores=N_CORES):
    """Host-side shard + permute + fp8 hi/lo split. Returns in_maps."""
    n_tn = T // tb
    n_dk = d // 128
    n_fm = f // 128
    n_dm = d // 128

    tokens = np.ascontiguousarray(np.asarray(x, dtype=np.float32).reshape(-1, d))

    def perm_w(W, n_rows):
        # [n_rows*128, K] -> [n_rows, 128(p), K/128(k), 128(m)]
        return np.ascontiguousarray(
            W.reshape(n_rows, 128, -1, 128).transpose(0, 3, 2, 1))

    wg_hi, wg_lo = _fp8_split(np.asarray(W_gate, np.float32) * W_SCALE)
    wu_hi, wu_lo = _fp8_split(np.asarray(W_up, np.float32) * W_SCALE)
    wd_hi, wd_lo = _fp8_split(np.asarray(W_down, np.float32) * W_SCALE)

    wgh_np = perm_w(wg_hi, n_fm)
    wgl_np = perm_w(wg_lo, n_fm)
    wuh_np = perm_w(wu_hi, n_fm)
    wul_np = perm_w(wu_lo, n_fm)
    wdh_np = perm_w(wd_hi, n_dm)
    wdl_np = perm_w(wd_lo, n_dm)

    in_maps = []
    for c in range(n_cores):
        xc = tokens[c * T:(c + 1) * T]  # [T, d]
        x_hi, x_lo = _fp8_split(xc)
        # [T, d] -> [n_tn, 128(p), n_dk(k), tb(t)]
        xh_np = np.ascontiguousarray(
            x_hi.reshape(n_tn, tb, n_dk, 128).transpose(0, 3, 2, 1))
        xl_np = np.ascontiguousarray(
            x_lo.reshape(n_tn, tb, n_dk, 128).transpose(0, 3, 2, 1))
        in_maps.append({
            "xh": xh_np, "xl": xl_np,
            "wgh": wgh_np, "wgl": wgl_np,
            "wuh": wuh_np, "wul": wul_np,
            "wdh": wdh_np, "wdl": wdl_np,
        })
    return in_maps


def _postprocess(results, T=T_CORE, d=D, n_cores=N_CORES):
    """y[dm, p, t] per core -> full [B, S, D] float32."""
    outs = []
    for c in range(n_cores):
        yc = results[c]["y"]  # [n_dm, 128, T]
        outs.append(yc.transpose(2, 0, 1).reshape(T, d))
    return np.concatenate(outs, axis=0)


def kernel(x, W_gate, W_up, W_down):
    import time

    if "nc" not in LAST_RUN:
        t0 = time.perf_counter()
        LAST_RUN["nc"] = build_module()
        LAST_RUN["build_s"] = time.perf_counter() - t0
    nc = LAST_RUN["nc"]

    t0 = time.perf_counter()
    in_maps = _prep_inputs(x, W_gate, W_up, W_down)
    LAST_RUN["prep_s"] = time.perf_counter() - t0

    t0 = time.perf_counter()
    res = run_bass_kernel_spmd(nc, in_maps, core_ids=list(range(N_CORES)))
    LAST_RUN["run_s"] = time.perf_counter() - t0
    LAST_RUN["results"] = res

    out = _postprocess(res.results)
    return out.reshape(B, S, D)



# revision 9
# speedup vs baseline: 1.4917x; 1.4917x over previous
"""LLaMA-style MLP (gate/up/silu/down) on 8 Trainium2 NeuronCores.

Strategy: data-parallel over tokens (8192 tokens -> 1024/core), fp8
matmuls in DoubleRow perf mode (2 fp8 contraction elements per PE pass,
0.5 cycles/row -> 4x bf16 matmul throughput) with fp32 PSUM
accumulation and no collectives.

Accuracy is recovered with a TWO-pass "alpha-mix" error compensation
(vs the classic 3-pass hi/lo scheme): every operand V is encoded as

    V_hi = fp8(V)
    V_mx = fp8(sqrt(a)*V_hi + (V - V_hi)/sqrt(a)),   a = 1/8

and each logical matmul W@X becomes two fp8 DoubleRow matmuls
accumulated in PSUM:

    W_hi@X_hi + W_mx@X_mx
      = (1+a)*W_hi@X_hi + W_hi@X_lo + W_lo@X_hi + W_lo@X_lo/a + O(eps*sqrt(a))

so dividing the PSUM result by (1+a) recovers W@X with all first-order
quantization corrections included.  The residual error terms are the
a-suppressed re-quantization noise of the mix operands (~eps*sqrt(a)),
the a-fold overcounted lo*lo term (~eps^2/a) and the (1+a) under-scaled
corrections (~a*eps); at a=1/8 these balance to ~0.8% per matmul,
~1.3e-2 end to end (vs 2e-2 budget; the 3-pass scheme gave 2.8e-3 at
1.5x the PE cost).  PE cost: 2 passes * 0.25 cyc per 128x128 MAC tile
= 2/3 of the 3-pass scheme -> ~1.76 ms of matmul at 2.4 GHz.

Weights are pre-scaled by 64 on the host so their magnitudes sit in
e4m3's normal range; the scale and the 1/(1+a) correction are divided
back out on-device (SiLU input scale, down-proj output scale).

Layouts (host pre-permutes; partition dim first, contraction subtiles
paired for DoubleRow's [p, 2, free] operand shape):

  x   -> xh/x2 [n_tn, 128, D/128, TB]    x*[tn,p,k,t] = encoding of x[tok, k*128+p]
  Wg  -> wgh/wg2 [F/128, 128, D/128, 128]  = encoding of 64*Wg[fm*128+m, k*128+p]
  Wu  -> wuh/wu2 (same layout)
  Wd  -> wdh/wd2 [D/128, 128, F/128, 128]  = encoding of 64*Wd[dm*128+m, k*128+p]
  out <- y [D/128, 128, T] f32           y[dm,p,t] = out[tok, dm*128+p]

Per 512-token block: gate/up PSUM chains contract D in 16 pairs x 2
passes, SiLU(psg/(64(1+a))) on the scalar engine, then on the vector
engine hf = H/sqrt(a) (fused scale), hh = fp8(sqrt(a)*hf) and
h2 = fp8(hf + (sqrt(a)-1/sqrt(a))*hh), and the down projection
contracts F in 43 pairs x 2 passes.

Overlap details:
 - The first slab's chains are emitted pass-major (all (Wh,xh) chunks,
   then (Wmx,xmx)) with the warmup DMAs interleaved in the same order,
   so the PE starts as soon as Wh+xh land instead of waiting for all
   four operand tiles.
 - Dummy DoubleRow matmuls on memset scratch keep the PE continuously
   busy across the initial DMA window and the in-flight-x2 gap
   (N_WARM0/N_WARM1): the tensor engine's p-state ramp (half speed for
   3us after any idle) is spent on free work, never on real chains.
 - The next token block's x tiles are prefetched behind the first wd
   pair of the previous down phase.
 - The kernel's final down chain is split into four quarter-token
   chains so y copies/stores overlap the remaining matmuls instead of
   trailing the kernel.
"""

import os
import sys

sys.path.insert(0, "/opt/trn_rl_repo")

import math
from contextlib import ExitStack

import numpy as np
import ml_dtypes

import concourse.bass as bass  # noqa: F401
import concourse.tile as tile
import concourse.mybir as mybir
from concourse import bacc
from concourse.bass_utils import run_bass_kernel_spmd

BF16 = mybir.dt.bfloat16
F32 = mybir.dt.float32
FP8 = mybir.dt.float8e4
NP_FP8 = ml_dtypes.float8_e4m3

# Problem shape (hardcoded per the task contract).
B, S, D, F = 4, 2048, 4096, 11008
N_CORES = 8
T_CORE = (B * S) // N_CORES  # tokens per core
TB = 512                     # token block (one PSUM bank of fp32)
W_SCALE = 64.0               # host-side weight scale (power of 2)
ALPHA = 0.125                # mix strength of the 2-pass compensation
SQA = math.sqrt(ALPHA)

# Warmup PE-priming: dummy DoubleRow matmuls on memset scratch keep the
# tensor engine continuously busy while the first DMAs land, so the p-state
# ramp (2x-slow first 3us after any idle) is spent on free work, not real
# chains.  Counts tuned against the timeline sim.
N_WARM0 = 280  # kernel start -> first real matmul
N_WARM1 = 48   # last-operand gap inside the first pass-major chain

DR = mybir.MatmulPerfMode.DoubleRow

LAST_RUN = {}


def build_module(T=T_CORE, tb=TB, d=D, f=F):
    """Build the single-core Bass module (same program on all 8 cores)."""
    n_tn = T // tb          # 2 token blocks
    n_dk = d // 128         # 32 contraction subtiles for gate/up
    n_fm = f // 128         # 86 F slabs
    n_fk = f // 128         # 86 contraction subtiles for down
    n_dm = d // 128         # 32 D slabs

    nc = bacc.Bacc("TRN2", target_bir_lowering=False, debug=False)
    xh = nc.dram_tensor("xh", [n_tn, 128, n_dk, tb], FP8, kind="ExternalInput").ap()
    x2 = nc.dram_tensor("x2", [n_tn, 128, n_dk, tb], FP8, kind="ExternalInput").ap()
    wgh = nc.dram_tensor("wgh", [n_fm, 128, n_dk, 128], FP8, kind="ExternalInput").ap()
    wg2 = nc.dram_tensor("wg2", [n_fm, 128, n_dk, 128], FP8, kind="ExternalInput").ap()
    wuh = nc.dram_tensor("wuh", [n_fm, 128, n_dk, 128], FP8, kind="ExternalInput").ap()
    wu2 = nc.dram_tensor("wu2", [n_fm, 128, n_dk, 128], FP8, kind="ExternalInput").ap()
    wdh = nc.dram_tensor("wdh", [n_dm, 128, n_fk, 128], FP8, kind="ExternalInput").ap()
    wd2 = nc.dram_tensor("wd2", [n_dm, 128, n_fk, 128], FP8, kind="ExternalInput").ap()
    y = nc.dram_tensor("y", [n_dm, 128, T], BF16, kind="ExternalOutput").ap()

    inv = 1.0 / (W_SCALE * (1.0 + ALPHA))  # undo weight scale + (1+a)

    with tile.TileContext(nc) as tc, ExitStack() as ctx:
        xpool = ctx.enter_context(tc.tile_pool(name="x", bufs=1))
        wpool = ctx.enter_context(tc.tile_pool(name="w", bufs=2))
        wdpool = ctx.enter_context(tc.tile_pool(name="wdp", bufs=2))
        hpool = ctx.enter_context(tc.tile_pool(name="h", bufs=1))
        spool = ctx.enter_context(tc.tile_pool(name="s", bufs=2))
        fpool = ctx.enter_context(tc.tile_pool(name="hf", bufs=2))
        ypool = ctx.enter_context(tc.tile_pool(name="y", bufs=2))
        psum = ctx.enter_context(tc.tile_pool(name="psum", bufs=4, space="PSUM"))
        psumy = ctx.enter_context(tc.tile_pool(name="psumy", bufs=2, space="PSUM"))
        zpool = ctx.enter_context(tc.tile_pool(name="z", bufs=1))

        n_pairs = n_dk // 2

        zw_sb = zpool.tile([128, 2, 128], FP8, tag="zw")
        nc.vector.memset(zw_sb[:], 0)
        zx_sb = zpool.tile([128, 2, 128], FP8, tag="zx")
        nc.vector.memset(zx_sb[:], 0)

        def emit_warm(n):
            """n small dummy DoubleRow matmuls into scratch PSUM (borrows a
            psy-tagged bank, which is otherwise idle during warmup)."""
            if n <= 0:
                return
            psz = psumy.tile([128, tb], F32, tag="psy")
            for i in range(n):
                nc.tensor.matmul(
                    psz[:, 0:128], zw_sb[:], zx_sb[:],
                    start=(i == 0), stop=(i == n - 1), perf_mode=DR,
                )

        def emit_proj(ps, wh_sb, w2_sb, xh_sb, x2_sb, pass_major, warm_fill=0):
            """One 2-pass DoubleRow accumulation chain into `ps`.

            chunk-major needs all 4 operand tiles before the first matmul;
            pass-major orders the groups (Wh,xh), (Wmx,xmx) so the chain can
            start as soon as Wh and xh have landed — used for the first
            slab, where the chain start gates on the initial DMAs.
            """
            groups = [(wh_sb, xh_sb), (w2_sb, x2_sb)]
            if pass_major:
                for gi, (w_sb, x_sb) in enumerate(groups):
                    if gi == 1:
                        # the last operand's transfer may still be in flight;
                        # busy-wait on scratch so the p-state ramp isn't reset
                        emit_warm(warm_fill)
                    for c in range(n_pairs):
                        sl = slice(2 * c, 2 * c + 2)
                        nc.tensor.matmul(
                            ps[:], w_sb[:, sl], x_sb[:, sl],
                            start=(gi == 0 and c == 0),
                            stop=(gi == 1 and c == n_pairs - 1),
                            perf_mode=DR,
                        )
            else:
                for c in range(n_pairs):
                    sl = slice(2 * c, 2 * c + 2)
                    nc.tensor.matmul(
                        ps[:], wh_sb[:, sl], xh_sb[:, sl],
                        start=(c == 0), stop=False, perf_mode=DR,
                    )
                    nc.tensor.matmul(
                        ps[:], w2_sb[:, sl], x2_sb[:, sl],
                        start=False, stop=(c == n_pairs - 1), perf_mode=DR,
                    )

        next_x = None
        next_w0 = None
        for tn in range(n_tn):
            first = tn == 0
            if first:
                # Interleave the first slab's weight DMAs with the x DMAs in
                # the order the pass-major chain consumes them.
                wgh_sb = wpool.tile([128, n_dk, 128], FP8, tag="wgh")
                nc.sync.dma_start(wgh_sb[:], wgh[0])
                xh_sb = xpool.tile([128, n_dk, tb], FP8, tag="xh")
                hk = n_dk // 2
                nc.sync.dma_start(xh_sb[:, :hk], xh[tn, :, :hk])
                nc.sync.dma_start(xh_sb[:, hk:], xh[tn, :, hk:])
                wg2_sb = wpool.tile([128, n_dk, 128], FP8, tag="wg2")
                nc.sync.dma_start(wg2_sb[:], wg2[0])
                x2_sb = xpool.tile([128, n_dk, tb], FP8, tag="x2")
                nc.sync.dma_start(x2_sb[:, :hk], x2[tn, :, :hk])
                nc.sync.dma_start(x2_sb[:, hk:], x2[tn, :, hk:])
                wuh_sb = wpool.tile([128, n_dk, 128], FP8, tag="wuh")
                nc.sync.dma_start(wuh_sb[:], wuh[0])
                wu2_sb = wpool.tile([128, n_dk, 128], FP8, tag="wu2")
                nc.sync.dma_start(wu2_sb[:], wu2[0])
                w0 = (wgh_sb, wg2_sb, wuh_sb, wu2_sb)
                emit_warm(N_WARM0)
            else:
                # Issued during the previous block's down projection so the
                # transfers overlap stage B instead of stalling this block.
                xh_sb, x2_sb = next_x
                w0 = next_w0

            hh_sb = hpool.tile([128, n_fm, tb], FP8, tag="hh")
            h2_sb = hpool.tile([128, n_fm, tb], FP8, tag="h2")

            # Stage A: gate/up projection + silu + mul + fp8 mix encoding,
            # one 128-row slab of F at a time.
            for fm in range(n_fm):
                if fm == 0:
                    wgh_sb, wg2_sb, wuh_sb, wu2_sb = w0
                else:
                    wgh_sb = wpool.tile([128, n_dk, 128], FP8, tag="wgh")
                    nc.sync.dma_start(wgh_sb[:], wgh[fm])
                    wg2_sb = wpool.tile([128, n_dk, 128], FP8, tag="wg2")
                    nc.sync.dma_start(wg2_sb[:], wg2[fm])
                    wuh_sb = wpool.tile([128, n_dk, 128], FP8, tag="wuh")
                    nc.sync.dma_start(wuh_sb[:], wuh[fm])
                    wu2_sb = wpool.tile([128, n_dk, 128], FP8, tag="wu2")
                    nc.sync.dma_start(wu2_sb[:], wu2[fm])

                tm = first and fm == 0
                psg = psum.tile([128, tb], F32, tag="ps")
                emit_proj(psg, wgh_sb, wg2_sb, xh_sb, x2_sb, tm,
                          warm_fill=N_WARM1 if tm else 0)
                psu = psum.tile([128, tb], F32, tag="ps")
                emit_proj(psu, wuh_sb, wu2_sb, xh_sb, x2_sb, False)

                # sg = silu(psg * inv)  [bf16]
                sg = spool.tile([128, tb], BF16, tag="sg")
                nc.scalar.activation(
                    sg[:], psg[:], mybir.ActivationFunctionType.Silu, scale=inv
                )
                # hf = H/sqrt(a) = (psu*inv)*sg/sqrt(a)  [f32]
                hf = fpool.tile([128, tb], F32, tag="hf")
                nc.vector.scalar_tensor_tensor(
                    hf[:], psu[:], inv / SQA, sg[:],
                    mybir.AluOpType.mult, mybir.AluOpType.mult,
                )
                # hh = fp8(sqrt(a)*hf) = fp8(H)
                nc.vector.tensor_scalar_mul(hh_sb[:, fm], hf[:], SQA)
                # h2 = fp8(hf + (sqrt(a)-1/sqrt(a))*hh)
                #    = fp8(sqrt(a)*Hh + (H - Hh)/sqrt(a))
                nc.vector.scalar_tensor_tensor(
                    h2_sb[:, fm], hh_sb[:, fm], SQA - 1.0 / SQA, hf[:],
                    mybir.AluOpType.mult, mybir.AluOpType.add,
                )

            # Stage B: down projection, contracting over all of F.
            # wd tiles are split into a (44 ktiles) / b (42 ktiles) halves so
            # each DMA is ~2us and the ring-slot WAR dependency frees at the
            # reader's mid-chain instead of chain end — finer DMA pipelining.
            ka = 44
            for dm in range(n_dm):
                wdh_a = wdpool.tile([128, ka, 128], FP8, tag="wdh_a")
                nc.sync.dma_start(wdh_a[:], wdh[dm, :, :ka])
                wdh_b = wdpool.tile([128, n_fk - ka, 128], FP8, tag="wdh_b")
                nc.sync.dma_start(wdh_b[:], wdh[dm, :, ka:])
                wd2_a = wdpool.tile([128, ka, 128], FP8, tag="wd2_a")
                nc.sync.dma_start(wd2_a[:], wd2[dm, :, :ka])
                wd2_b = wdpool.tile([128, n_fk - ka, 128], FP8, tag="wd2_b")
                nc.sync.dma_start(wd2_b[:], wd2[dm, :, ka:])
                if tn + 1 < n_tn:
                    # Prefetch the next token block's x in 8 k-chunks spread
                    # over down slabs 1..8 (a single 4MB burst would
                    # head-of-line-block this stage's own wd transfers), then
                    # its first gate/up weight slab behind slabs 9..12 so the
                    # next stage-A starts without a DMA bubble.
                    if dm == 1:
                        nxh = xpool.tile([128, n_dk, tb], FP8, tag="xh")
                        nx2 = xpool.tile([128, n_dk, tb], FP8, tag="x2")
                        next_x = (nxh, nx2)
                    if 1 <= dm <= 8:
                        ck = n_dk // 8
                        ks = slice((dm - 1) * ck, dm * ck)
                        nc.sync.dma_start(next_x[0][:, ks], xh[tn + 1, :, ks])
                        nc.sync.dma_start(next_x[1][:, ks], x2[tn + 1, :, ks])
                    elif 9 <= dm <= 12:
                        wsrc = (wgh, wg2, wuh, wu2)[dm - 9]
                        wtag = ("wgh", "wg2", "wuh", "wu2")[dm - 9]
                        nw = wpool.tile([128, n_dk, 128], FP8, tag=wtag)
                        nc.sync.dma_start(nw[:], wsrc[0])
                        if dm == 9:
                            next_w0 = []
                        next_w0.append(nw)
                n_fpairs = n_fk // 2
                # Pass-major pair sequence over the split wd tiles: all
                # (wdh, hh) pairs first so the chain starts once wdh_a
                # lands, with wd2_* transfers still in flight.
                seq = []
                for wa, wb, h in ((wdh_a, wdh_b, hh_sb), (wd2_a, wd2_b, h2_sb)):
                    for c in range(n_fpairs):
                        g0 = 2 * c
                        if g0 + 2 <= ka:
                            w, wsl = wa, slice(g0, g0 + 2)
                        else:
                            w, wsl = wb, slice(g0 - ka, g0 - ka + 2)
                        seq.append((w, wsl, h, slice(g0, g0 + 2)))
                if tn == n_tn - 1 and dm == n_dm - 1:
                    # Last chain of the kernel: split into eight token-slice
                    # chains so earlier slices' y copy + store overlap the
                    # later slices' matmuls instead of trailing the kernel.
                    nsp = 8
                    for part in range(nsp):
                        ts = slice(part * (tb // nsp), (part + 1) * (tb // nsp))
                        psyh = psumy.tile([128, tb // nsp], F32, tag="psyh")
                        for i, (w, wsl, h, hsl) in enumerate(seq):
                            nc.tensor.matmul(
                                psyh[:], w[:, wsl], h[:, hsl, ts],
                                start=(i == 0), stop=(i == len(seq) - 1),
                                perf_mode=DR,
                            )
                        y_sb = ypool.tile([128, tb // nsp], BF16, tag="yh")
                        nc.vector.tensor_scalar_mul(y_sb[:], psyh[:], inv)
                        nc.sync.dma_start(
                            y[dm, :, tn * tb + part * (tb // nsp):
                                     tn * tb + (part + 1) * (tb // nsp)],
                            y_sb[:])
                    continue
                psy = psumy.tile([128, tb], F32, tag="psy")
                for i, (w, wsl, h, hsl) in enumerate(seq):
                    nc.tensor.matmul(
                        psy[:], w[:, wsl], h[:, hsl],
                        start=(i == 0), stop=(i == len(seq) - 1), perf_mode=DR,
                    )
                y_sb = ypool.tile([128, tb], BF16, tag="y")
                nc.vector.tensor_scalar_mul(y_sb[:], psy[:], inv)
                nc.sync.dma_start(y[dm, :, tn * tb:(tn + 1) * tb], y_sb[:])

    nc.compile()
    return nc


def _fp8_mix_enc(a):
    """Encode float32 array as (hi, mix) float8_e4m3 parts:
    hi = fp8(a), mix = fp8(sqrt(alpha)*hi + (a - hi)/sqrt(alpha))."""
    hi = a.astype(NP_FP8)
    hf = hi.astype(np.float32)
    mx = (np.float32(SQA) * hf + (a - hf) * np.float32(1.0 / SQA)).astype(NP_FP8)
    return hi, mx


def _prep_inputs(x, W_gate, W_up, W_down, T=T_CORE, tb=TB, d=D, f=F,
                 n_cores=N_CORES):
    """Host-side shard + permute + fp8 mix encoding. Returns in_maps."""
    n_tn = T // tb
    n_dk = d // 128
    n_fm = f // 128
    n_dm = d // 128

    tokens = np.ascontiguousarray(np.asarray(x, dtype=np.float32).reshape(-1, d))

    def perm_w(W, n_rows):
        # [n_rows*128, K] -> [n_rows, 128(p), K/128(k), 128(m)]
        return np.ascontiguousarray(
            W.reshape(n_rows, 128, -1, 128).transpose(0, 3, 2, 1))

    wg_hi, wg_mx = _fp8_mix_enc(np.asarray(W_gate, np.float32) * W_SCALE)
    wu_hi, wu_mx = _fp8_mix_enc(np.asarray(W_up, np.float32) * W_SCALE)
    wd_hi, wd_mx = _fp8_mix_enc(np.asarray(W_down, np.float32) * W_SCALE)

    wgh_np = perm_w(wg_hi, n_fm)
    wg2_np = perm_w(wg_mx, n_fm)
    wuh_np = perm_w(wu_hi, n_fm)
    wu2_np = perm_w(wu_mx, n_fm)
    wdh_np = perm_w(wd_hi, n_dm)
    wd2_np = perm_w(wd_mx, n_dm)

    in_maps = []
    for c in range(n_cores):
        xc = tokens[c * T:(c + 1) * T]  # [T, d]
        x_hi, x_mx = _fp8_mix_enc(xc)
        # [T, d] -> [n_tn, 128(p), n_dk(k), tb(t)]
        xh_np = np.ascontiguousarray(
            x_hi.reshape(n_tn, tb, n_dk, 128).transpose(0, 3, 2, 1))
        x2_np = np.ascontiguousarray(
            x_mx.reshape(n_tn, tb, n_dk, 128).transpose(0, 3, 2, 1))
        in_maps.append({
            "xh": xh_np, "x2": x2_np,
            "wgh": wgh_np, "wg2": wg2_np,
            "wuh": wuh_np, "wu2": wu2_np,
            "wdh": wdh_np, "wd2": wd2_np,
        })
    return in_maps


def _postprocess(results, T=T_CORE, d=D, n_cores=N_CORES):
    """y[dm, p, t] per core (bf16) -> full [B, S, D] float32."""
    outs = []
    for c in range(n_cores):
        yc = np.asarray(results[c]["y"]).astype(np.float32)  # [n_dm, 128, T]
        outs.append(yc.transpose(2, 0, 1).reshape(T, d))
    return np.concatenate(outs, axis=0)


def kernel(x, W_gate, W_up, W_down):
    import time

    if "nc" not in LAST_RUN:
        t0 = time.perf_counter()
        LAST_RUN["nc"] = build_module()
        LAST_RUN["build_s"] = time.perf_counter() - t0
    nc = LAST_RUN["nc"]

    t0 = time.perf_counter()
    in_maps = _prep_inputs(x, W_gate, W_up, W_down)
    LAST_RUN["prep_s"] = time.perf_counter() - t0

    t0 = time.perf_counter()
    res = run_bass_kernel_spmd(nc, in_maps, core_ids=list(range(N_CORES)))
    LAST_RUN["run_s"] = time.perf_counter() - t0
    LAST_RUN["results"] = res

    out = _postprocess(res.results)
    return out.reshape(B, S, D)
